# revision 6
# baseline (speedup 1.0000x reference)
"""CutCrossEntropyLoss (sampled softmax, 512 noise + 1 target per token) on 8 trn2 cores.

The metric here is wall-clock per run including host->device upload over the
axon tunnel (~50 MB/s), so the design minimizes uploaded bytes (the baseline
replicated a 77MB bf16 weight to all 8 cores; this uploads ~46MB total):
 - Vocab-parallel: shard the 50257-row classifier over the 8 cores (6288 rows
   each, fp8_e4m3, uploaded exactly once, resident in SBUF).
 - hidden_states: upload only each core's 128-token shard (fp8), AllGather the
   full 1024 tokens on device through a DRAM bounce buffer.
 - Each core computes the full logits block h @ Wc^T for its shard: 8 token
   groups x 13 vocab tiles x 6 accumulating K=128 matmuls into PSUM, drained
   to an SBUF stage tile [128 tokens, 6288] f32.
 - Sampled-softmax reductions per token need the 513 sampled logits (with
   multiplicity) out of the shard.  Host packs, per (token, 1572-vocab
   subrange), the distinct sampled local indices with counts c; the device
   expands them to dense rows C [128, 6288] bf16 with gpsimd.local_scatter
   (zero background, -1 pads ignored), plus a target one-hot row T from a
   2-slot index list (constant 1.0 data, no upload).
 - Per token group: rowmax m (unmasked shard max is a valid stabilizer), then
   xt = sum(T*x), scx = sum(C*x), exp in place via scalar activation with
   bias=-m, Z = sum(C*exp(x-m)).  Products are parked in-place in the bf16
   C/T tiles; sums reduce in f32.  Each core returns [128, 8 x (negmax, Z,
   scx, xt)] f32.
 - Host combine (tiny): per token M = max_c m_c, Ztot = sum_c Z_c*exp(m_c-M),
   loss = M + log(Ztot) - NPROB*sum_c scx_c - (0.9-NPROB)*sum_c xt_c; mean.
   (loss = lse - 0.9*lt - NPROB*(ssum - lt); target is always in the sampled
   multiset so its count rides in C.)
"""
import sys

sys.path.insert(0, "/opt/trn_rl_repo")

import numpy as np
import ml_dtypes

H = 768
KC = 6  # H / 128
V = 50257
NTOK = 1024
SAMPLE = 512
NCORES = 8

SH = 6288  # vocab rows per core shard (8 * 6288 = 50304 >= 50257; pad rows are zero)
NSUB = 4  # local_scatter subranges per shard
SUBW = SH // NSUB  # 1572 (< 2048 gpsimd local_scatter limit)
NG = 8  # token groups of 128
TPG = 128
KSLOT = 36  # packed (idx, val) slots per (token, subrange); actual max is 32
NT = (SH + 511) // 512  # 13 vocab tiles (12 x 512 + 144)

LS = 0.1
NPROB = LS / SAMPLE

_CACHE = {}


def _build_bass():
    import concourse.bacc as bacc
    import concourse.mybir as mybir
    from concourse import tile

    nc = bacc.Bacc("TRN2", debug=False, num_devices=NCORES)
    f32 = mybir.dt.float32
    bf16 = mybir.dt.bfloat16
    fp8 = mybir.dt.float8e4
    i16 = mybir.dt.int16
    AX = mybir.AxisListType.X
    OP = mybir.AluOpType
    ACTF = mybir.ActivationFunctionType

    wt = nc.dram_tensor("wt", [128, KC * SH], fp8, kind="ExternalInput")
    hts = nc.dram_tensor("hts", [128, KC * TPG], fp8, kind="ExternalInput")
    cidx = nc.dram_tensor("cidx", [128, NG * NSUB * KSLOT], i16, kind="ExternalInput")
    cval = nc.dram_tensor("cval", [128, NG * NSUB * KSLOT], bf16, kind="ExternalInput")
    tidx = nc.dram_tensor("tidx", [128, NG * NSUB * 2], i16, kind="ExternalInput")
    out = nc.dram_tensor("out", [128, NG * 4], f32, kind="ExternalOutput")

    with tile.TileContext(nc) as tc:
        with (
            tc.tile_pool(name="const", bufs=1) as cpool,
            tc.tile_pool(name="cw", bufs=2) as cwpool,
            tc.tile_pool(name="stage", bufs=2) as spool,
            tc.tile_pool(name="ps", bufs=4, space="PSUM") as ppool,
            tc.tile_pool(name="work", bufs=1) as wpool,
            tc.tile_pool(name="dram", bufs=1, space="DRAM") as dpool,
        ):
            # --- gather all 1024 tokens' h^T from the 8 per-core shards ---
            hb_in = dpool.tile([128, KC * TPG], fp8)
            hb_out = dpool.tile([NCORES, 128, KC * TPG], fp8)
            nc.gpsimd.dma_start(hb_in[:], hts[:])
            nc.gpsimd.collective_compute(
                "AllGather",
                mybir.AluOpType.bypass,
                replica_groups=[list(range(NCORES))],
                ins=[hb_in[:].opt()],
                outs=[hb_out[:].opt()],
            )
            ht_t = cpool.tile([128, KC, NTOK], fp8)
            for g in range(NG):
                nc.sync.dma_start(
                    out=ht_t[:, :, g * TPG : (g + 1) * TPG],
                    in_=hb_out[g].rearrange("p (c t) -> p c t", c=KC),
                )

            wt_t = cpool.tile([128, KC, SH], fp8)
            nc.sync.dma_start(out=wt_t[:], in_=wt[:].rearrange("p (c v) -> p c v", c=KC))
            cidx_t = cpool.tile([128, NG, NSUB, KSLOT], i16)
            nc.sync.dma_start(
                out=cidx_t[:],
                in_=cidx[:].rearrange("p (g s k) -> p g s k", g=NG, s=NSUB),
            )
            cval_t = cpool.tile([128, NG, NSUB, KSLOT], bf16)
            nc.sync.dma_start(
                out=cval_t[:],
                in_=cval[:].rearrange("p (g s k) -> p g s k", g=NG, s=NSUB),
            )
            tidx_t = cpool.tile([128, NG, NSUB, 2], i16)
            nc.sync.dma_start(
                out=tidx_t[:],
                in_=tidx[:].rearrange("p (g s k) -> p g s k", g=NG, s=NSUB),
            )
            ones_t = cpool.tile([128, 2], bf16)
            nc.vector.memset(ones_t[:], 1.0)

            outt = wpool.tile([128, NG * 4], f32)

            for g in range(NG):
                C = cwpool.tile([128, SH], bf16, tag="C")
                T = cwpool.tile([128, SH], bf16, tag="T")
                for s in range(NSUB):
                    nc.gpsimd.local_scatter(
                        out_ap=C[:, s * SUBW : (s + 1) * SUBW],
                        data_ap=cval_t[:, g, s, :],
                        idxs_ap=cidx_t[:, g, s, :],
                        channels=128,
                        num_elems=SUBW,
                        num_idxs=KSLOT,
                    )
                    nc.gpsimd.local_scatter(
                        out_ap=T[:, s * SUBW : (s + 1) * SUBW],
                        data_ap=ones_t[:],
                        idxs_ap=tidx_t[:, g, s, :],
                        channels=128,
                        num_elems=SUBW,
                        num_idxs=2,
                    )

                stage = spool.tile([128, SH], f32, tag="st")
                for nt in range(NT):
                    w = min(512, SH - nt * 512)
                    ps = ppool.tile([128, 512], f32, tag="ps")
                    for c in range(KC):
                        nc.tensor.matmul(
                            out=ps[:, :w],
                            lhsT=ht_t[:, c, g * TPG : (g + 1) * TPG],
                            rhs=wt_t[:, c, nt * 512 : nt * 512 + w],
                            start=(c == 0),
                            stop=(c == KC - 1),
                        )
                    nc.scalar.copy(out=stage[:, nt * 512 : nt * 512 + w], in_=ps[:, :w])

                negmax = outt[:, 4 * g : 4 * g + 1]
                nc.vector.tensor_reduce(
                    out=negmax, in_=stage[:], axis=AX, op=OP.max, negate=True
                )
                # xt = sum(T * x); product parked in T (T becomes scratch after)
                nc.vector.tensor_tensor(out=T[:], in0=stage[:], in1=T[:], op=OP.mult)
                nc.vector.tensor_reduce(
                    out=outt[:, 4 * g + 3 : 4 * g + 4], in_=T[:], axis=AX, op=OP.add
                )
                # scx = sum(C * x); product parked in scratch T
                nc.vector.tensor_tensor(out=T[:], in0=stage[:], in1=C[:], op=OP.mult)
                nc.vector.tensor_reduce(
                    out=outt[:, 4 * g + 2 : 4 * g + 3], in_=T[:], axis=AX, op=OP.add
                )
                # stage <- exp(stage - max)
                nc.scalar.activation(
                    out=stage[:], in_=stage[:], func=ACTF.Exp, bias=negmax
                )
                # Z = sum(C * exp); product parked in C
                nc.vector.tensor_tensor(out=C[:], in0=stage[:], in1=C[:], op=OP.mult)
                nc.vector.tensor_reduce(
                    out=outt[:, 4 * g + 1 : 4 * g + 2], in_=C[:], axis=AX, op=OP.add
                )

            nc.sync.dma_start(out=out[:], in_=outt[:])

    nc.compile()
    return nc


def _prep_inputs(hidden_states, weight, target, noise_indx):
    h = np.asarray(hidden_states, np.float32).reshape(NTOK, H)
    W = np.asarray(weight, np.float32)
    tgt = np.asarray(target).reshape(NTOK).astype(np.int64)
    nz = np.asarray(noise_indx).astype(np.int64)
    fp8 = ml_dtypes.float8_e4m3
    bf16 = ml_dtypes.bfloat16

    # h^T split into 6 chunks of 128 h-dims -> [128, KC, NTOK]; core c uploads
    # only its 128-token block (AllGathered on device)
    htc = np.ascontiguousarray(
        h.T.reshape(KC, 128, NTOK).transpose(1, 0, 2)
    )  # [128, KC, NTOK] f32

    # Packed sparse (local idx, count) per (token, core, subrange)
    ids = np.concatenate([nz, tgt[:, None]], axis=1)  # [NTOK, 513]
    keys = (np.arange(NTOK, dtype=np.int64)[:, None] * (SH * NCORES) + ids).ravel()
    uk, cnt = np.unique(keys, return_counts=True)
    n_u = uk // (SH * NCORES)
    id_u = uk % (SH * NCORES)
    core_u = id_u // SH
    loc_u = id_u % SH
    sub_u = loc_u // SUBW
    lloc_u = loc_u % SUBW

    # slot index within each (token, core, subrange) run (uk is sorted)
    grp = (n_u * NCORES + core_u) * NSUB + sub_u
    starts = np.flatnonzero(np.diff(grp, prepend=-1))
    runid = np.cumsum(np.isin(np.arange(grp.shape[0]), starts)) - 1
    slot = np.arange(grp.shape[0]) - starts[runid]
    assert slot.max() < KSLOT, f"slot overflow: {slot.max()}"

    IDX = np.full((NTOK, NCORES, NSUB, KSLOT), -1, np.int16)
    CV = np.zeros((NTOK, NCORES, NSUB, KSLOT), np.float32)
    IDX[n_u, core_u, sub_u, slot] = lloc_u
    CV[n_u, core_u, sub_u, slot] = cnt

    TIDX = np.full((NTOK, NCORES, NSUB, 2), -1, np.int16)
    tcore = tgt // SH
    tloc = tgt % SH
    TIDX[np.arange(NTOK), tcore, tloc // SUBW, 0] = (tloc % SUBW).astype(np.int16)

    def percore(a, width):
        # token t = g*128 + p  ->  [128, NG * NSUB * width]
        return np.ascontiguousarray(
            a.reshape(NG, TPG, NSUB, width).transpose(1, 0, 2, 3)
        ).reshape(128, NG * NSUB * width)

    in_maps = []
    for c in range(NCORES):
        lo, hi = c * SH, min((c + 1) * SH, V)
        Wc = np.zeros((SH, H), np.float32)
        Wc[: hi - lo] = W[lo:hi]
        wtc = np.ascontiguousarray(
            Wc.T.reshape(KC, 128, SH).transpose(1, 0, 2)
        ).reshape(128, KC * SH).astype(fp8)
        htsc = np.ascontiguousarray(htc[:, :, c * TPG : (c + 1) * TPG]).reshape(
            128, KC * TPG
        ).astype(fp8)
        in_maps.append(
            {
                "wt": wtc,
                "hts": htsc,
                "cidx": percore(IDX[:, c], KSLOT),
                "cval": percore(CV[:, c], KSLOT).astype(bf16),
                "tidx": percore(TIDX[:, c], 2),
            }
        )
    return in_maps


def _combine(results):
    # per core: [128, NG*4] -> token t = g*128 + p at [p, 4g:4g+4]
    nm = np.stack(
        [np.asarray(r["out"], np.float64).reshape(128, NG, 4) for r in results]
    )  # [NCORES, 128, NG, 4]
    m = -nm[..., 0]
    Z = nm[..., 1]
    scx = nm[..., 2].sum(axis=0)
    xt = nm[..., 3].sum(axis=0)
    M = m.max(axis=0)  # [128, NG]
    Ztot = (Z * np.exp(m - M[None])).sum(axis=0)
    loss = M + np.log(Ztot) - NPROB * scx - (0.9 - NPROB) * xt
    return np.float32(loss.mean())


def kernel(hidden_states, weight, target, noise_indx):
    from concourse.bass_utils import run_bass_kernel_spmd

    if "nc" not in _CACHE:
        _CACHE["nc"] = _build_bass()
    nc = _CACHE["nc"]
    in_maps = _prep_inputs(hidden_states, weight, target, noise_indx)
    res = run_bass_kernel_spmd(nc, in_maps, core_ids=list(range(NCORES)))
    return _combine(res.results)
